# revision 42
# baseline (speedup 1.0000x reference)
"""Bilinear sampler (flow warp) on 8 Trainium2 NeuronCores.

reference semantics (per batch b, f32):
  xn = 0.3*flow_x + linspace(-1,1,W)[j];  x = (xn + 1) * W/2
  yn = 0.3*flow_y + linspace(-1,1,H)[i];  y = (yn + 1) * H/2
  out = bilinear sample of image at (y, x). When x is outside [0, W-1)
  or y outside [0, H-1), the reference's clipped corner indices
  coincide and the weights cancel exactly, so out = 0 there.

Strategy: pure data parallel, 2 images per core.

The image is host-repacked into "entries" of 2 rows x 4 cols x 32 ch
(fp16, 512B): entry (par, hp, u) = rows (2*hp+par, 2*hp+par+1), cols
(2*u .. 2*u+3). For any corner (y0, x0) the entry
  idx = par*16384 + hp*128 + u,  par = y0&1, hp = y0>>1, u = x0>>1
contains the full 2x2 bilinear neighborhood; idx <= 32767 fits the
int16 index of gpsimd.dma_gather, and ONE 512B gather descriptor per
output pixel fetches all 4 corners.

Per-pixel weights are stored PAIR-DUPLICATED ([..., slot, 2] fp16) so
the broadcast multiply against the gathered tile has a packed
innermost dim on every operand and runs in the DVE 2x perf mode. Each
tile's pixel range is split between DVE and GpSimd (Pool), each engine
handling its range end-to-end (multiplies, folds, its output slice) so
neither stalls on the other. Output is written fp16 and upcast on the
host.

The index/weight computation is emitted in small chunks interleaved
between tile pumps: each chunk fills the idle gap DVE has while the
DMA-bound gather stream runs, so the next block's gather indices are
always ready just before its first gather issues.

Pixel layout: per-core pixel g = s*128 + p (p = partition). dma_gather
writes chunk i to partition i%128, so a tile of P*K pixels lands as
[128, K]; its int16 indices are consumed from [i%16, i//16], which the
kernel builds into a rewrapped index tile via DMA+DVE shuffles.
"""

import numpy as np

import concourse.bacc as bacc
import concourse.mybir as mybir
import concourse.tile as tile
from concourse import library_config
from concourse.bass_utils import run_bass_kernel_spmd

B, H, W, C = 16, 256, 256, 32
NCORES = 8
BPC = B // NCORES            # images per core
NPIX = BPC * H * W           # pixels per core (131072)
P = 128                      # partitions
SLOTS = NPIX // P            # free slots per partition (1024)
K = 64                       # pixel slots per gather tile
NT = SLOTS // K              # gather tiles per core (16)
KD = 55                      # pixel slots of each tile processed on DVE
                             # (the rest go to GpSimd/Pool)
IMG_PIX = H * W              # pixels per image (65536)
NE = 2 * (H // 2) * (W // 2)  # entries per image (32768)
ES = 2 * 4 * C               # elems per entry (256 fp16 = 512B)
HS = SLOTS // 2              # per-image slot count (512)
PH = 512                     # phase block size (slots); divides HS
NPH = SLOTS // PH            # phase blocks (4)
TPB = PH // K                # tiles per phase block (4)
LOOKAHEAD = 2                # gathers in flight ahead of processing

F32 = mybir.dt.float32
F16 = mybir.dt.float16
I32 = mybir.dt.int32
I16 = mybir.dt.int16
Alu = mybir.AluOpType


def _build_program():
    nc = bacc.Bacc("TRN2", target_bir_lowering=False, debug=False,
                   num_devices=NCORES)

    img = nc.dram_tensor("img", [BPC, NE, ES], F16, kind="ExternalInput").ap()
    flow = nc.dram_tensor("flow", [NPIX, 2], F32, kind="ExternalInput").ap()
    xbt = nc.dram_tensor("xbt", [P, 2], F32, kind="ExternalInput").ap()
    ybt = nc.dram_tensor("ybt", [P, HS // 2], F32, kind="ExternalInput").ap()
    out = nc.dram_tensor("out", [NPIX, C], F16, kind="ExternalOutput").ap()

    # flow/out are host-permuted to (p, s) order so these DMAs move
    # contiguous per-partition runs instead of tiny scatters
    out_v = out.rearrange("(p s) c -> p s c", p=P)
    flow_v = flow.rearrange("(p s) c -> p s c", p=P)

    with tile.TileContext(nc) as tc:
        with (
            tc.tile_pool(name="gather", bufs=3) as gpool,
            tc.tile_pool(name="outp", bufs=2) as opool,
            tc.tile_pool(name="idxp", bufs=2) as ipool,
            tc.tile_pool(name="wp", bufs=2) as wpool,
        ):
            nc.gpsimd.load_library(library_config.mlp)
            # per-phase-block ring tiles: gather indices and duplicated
            # weights live only from their phase block to its last tile
            blk = {}

            with (
                tc.tile_pool(name="tmp", bufs=1) as tpool,
                tc.tile_pool(name="tmp2", bufs=1) as t2pool,
            ):
                XB = t2pool.tile([P, 2], F32, tag="XB")
                YB = t2pool.tile([P, HS // 2], F32, tag="YB")
                IDXC = t2pool.tile([P, PH], I16, tag="IDXC")
                SC = t2pool.tile([16, 8, PH], I16, tag="SC")
                II = t2pool.tile([P, PH], I32, tag="II")
                # fp16 intermediates for the weight phase, slot-major so
                # every op reads/writes a packed innermost dim
                H16 = t2pool.tile([P, 8, PH], F16, tag="H16")
                W8t = t2pool.tile([P, 8, PH], F16, tag="W8t")
                nc.vector.memzero(W8t[:, 3:8:4, :])
                nc.sync.dma_start(out=XB[:], in_=xbt[:])
                nc.sync.dma_start(out=YB[:], in_=ybt[:])

                ts, tt, cp = (nc.vector.tensor_scalar,
                              nc.vector.tensor_tensor,
                              nc.vector.tensor_copy)

                def t(tag):
                    return tpool.tile([P, PH], F32, tag=tag, name=tag)[:]

                def phase_chunks(bi):
                    """Index+weight computation for phase block bi as a list
                    of small emission chunks (each ~1-3 us of DVE time)."""
                    s0 = bi * PH
                    F = t2pool.tile([P, PH, 2], F32, tag="F", name=f"F{bi}")
                    IDXG = ipool.tile([P, PH * 8], I16, tag="IDXG",
                                      name=f"IDXG{bi}")
                    W8d = wpool.tile([P, PH, 8, 2], F16, tag="W8d",
                                     name=f"W8d{bi}")
                    blk[bi] = (IDXG, W8d)
                    sl = slice(0, PH)

                    def c_flow():
                        nc.sync.dma_start(
                            out=F[:], in_=flow_v[:, s0:s0 + PH, :])
                    xp, yp = t("xp"), t("yp")
                    v, vt = t("v"), t("vt")
                    cx, cy = t("cx"), t("cy")
                    # tag-aliased successors (same buffers, fresh handles);
                    # hoisted so the chunk that writes and the chunk that
                    # reads share one tile object
                    uf, pmv = t("uf"), t("v")
                    hpf, pr, idf = t("hpf"), t("cx"), t("cy")
                    ii = II[:]

                    def c_coords():
                        # xb alternates xs[p], xs[128+p] along s; yb steps
                        # every other s; broadcast views over s-pairs
                        xb = XB[:].unsqueeze(1).broadcast_to([P, PH // 2, 2])
                        i0 = (s0 % HS) // 2
                        ybq = YB[:, i0:i0 + PH // 2]
                        yb = ybq.unsqueeze(2).broadcast_to([P, PH // 2, 2])
                        xp2 = xp.rearrange("p (a b) -> p a b", b=2)
                        yp2 = yp.rearrange("p (a b) -> p a b", b=2)
                        ts(out=xp, in0=F[:, :, 0], scalar1=0.3, scalar2=None,
                           op0=Alu.mult)
                        tt(out=xp2, in0=xp2, in1=xb, op=Alu.add)
                        ts(out=xp, in0=xp, scalar1=1.0,
                           scalar2=float(W) / 2.0, op0=Alu.add, op1=Alu.mult)
                        ts(out=yp, in0=F[:, :, 1], scalar1=0.3, scalar2=None,
                           op0=Alu.mult)
                        tt(out=yp2, in0=yp2, in1=yb, op=Alu.add)
                        ts(out=yp, in0=yp, scalar1=1.0,
                           scalar2=float(H) / 2.0, op0=Alu.add, op1=Alu.mult)

                    def c_valid():
                        # 0 <= x < W-1 and 0 <= y < H-1 (else out = 0);
                        # result lands in fp16 lane 3 of H16
                        ts(out=v, in0=xp, scalar1=0.0, scalar2=None,
                           op0=Alu.is_ge)
                        ts(out=vt, in0=xp, scalar1=float(W - 1), scalar2=None,
                           op0=Alu.is_lt)
                        tt(out=v, in0=v, in1=vt, op=Alu.mult)
                        ts(out=vt, in0=yp, scalar1=0.0, scalar2=None,
                           op0=Alu.is_ge)
                        tt(out=v, in0=v, in1=vt, op=Alu.mult)
                        ts(out=vt, in0=yp, scalar1=float(H - 1), scalar2=None,
                           op0=Alu.is_lt)
                        tt(out=H16[:, 3, sl], in0=v, in1=vt, op=Alu.mult)

                    def c_corners():
                        # round(x - 0.5) == floor(x) away from halves,
                        # matching the reference; clamp to [0, W-2]
                        xh = t("vt")
                        ts(out=xh, in0=xp, scalar1=-0.5, scalar2=None,
                           op0=Alu.add)
                        cp(out=ii, in_=xh)
                        cp(out=cx, in_=ii)
                        ts(out=cx, in0=cx, scalar1=0.0, scalar2=float(W - 2),
                           op0=Alu.max, op1=Alu.min)
                        yh = t("vt")
                        ts(out=yh, in0=yp, scalar1=-0.5, scalar2=None,
                           op0=Alu.add)
                        cp(out=ii, in_=yh)
                        cp(out=cy, in_=ii)
                        ts(out=cy, in0=cy, scalar1=0.0, scalar2=float(H - 2),
                           op0=Alu.max, op1=Alu.min)

                    def c_xdec():
                        # fractions -> fp16 lanes; u = x0>>1, pm = x0&1
                        tt(out=H16[:, 0, sl], in0=xp, in1=cx, op=Alu.subtract)
                        tt(out=H16[:, 1, sl], in0=yp, in1=cy, op=Alu.subtract)
                        ts(out=uf, in0=cx, scalar1=0.5, scalar2=-0.25,
                           op0=Alu.mult, op1=Alu.add)
                        cp(out=ii, in_=uf)
                        cp(out=uf, in_=ii)
                        ts(out=pmv, in0=uf, scalar1=-2.0, scalar2=None,
                           op0=Alu.mult)
                        tt(out=pmv, in0=pmv, in1=cx, op=Alu.add)
                        cp(out=H16[:, 2, sl], in_=pmv)

                    def c_ydec():
                        ts(out=hpf, in0=cy, scalar1=0.5, scalar2=-0.25,
                           op0=Alu.mult, op1=Alu.add)
                        cp(out=ii, in_=hpf)
                        cp(out=hpf, in_=ii)
                        ts(out=pr, in0=hpf, scalar1=-2.0, scalar2=None,
                           op0=Alu.mult)
                        tt(out=pr, in0=pr, in1=cy, op=Alu.add)
                        # entry index = pr*16384 + hpf*128 + u (int16-exact)
                        ts(out=idf, in0=pr, scalar1=float(NE // 2),
                           scalar2=None, op0=Alu.mult)
                        ts(out=hpf, in0=hpf, scalar1=float(W // 2),
                           scalar2=None, op0=Alu.mult)
                        tt(out=idf, in0=idf, in1=hpf, op=Alu.add)
                        tt(out=idf, in0=idf, in1=uf, op=Alu.add)
                        cp(out=IDXC[:, sl], in_=idf)

                    # rewrap indices for dma_gather: chunk i reads its int16
                    # index from [i%16, i//16] (replicated per 16-partition
                    # group); i = s_local*128 + p  =>
                    #   IDXG[16*rep + p%16, s*8 + p//16] = IDXC[p, s]
                    cols = slice(0, PH * 8)
                    dst = IDXG[0:16, cols].rearrange("p (s g) -> p s g", g=8)

                    def c_rw(g0):
                        def f():
                            for g in range(g0, g0 + 4):
                                nc.sync.dma_start(
                                    out=SC[:, g, :],
                                    in_=IDXC[16 * g:16 * (g + 1), :])
                                nc.vector.tensor_copy(out=dst[:, :, g],
                                                      in_=SC[:, g, :])
                        return f

                    def c_rep():
                        for rep in range(1, 8):
                            nc.sync.dma_start(
                                out=IDXG[16 * rep:16 * (rep + 1), cols],
                                in_=IDXG[0:16, cols])

                    def hx(i):
                        return H16[:, i, sl]

                    def wt(i):
                        return W8t[:, i, sl]

                    def c_wprep():
                        fx, fy, pm, v = hx(0), hx(1), hx(2), hx(3)
                        fx1, fy1, nm = hx(4), hx(5), hx(6)
                        ts(out=fx1, in0=fx, scalar1=-1.0, scalar2=1.0,
                           op0=Alu.mult, op1=Alu.add)
                        ts(out=fy1, in0=fy, scalar1=-1.0, scalar2=1.0,
                           op0=Alu.mult, op1=Alu.add)
                        ts(out=nm, in0=pm, scalar1=-1.0, scalar2=1.0,
                           op0=Alu.mult, op1=Alu.add)
                        # fold validity into the y factors (zeroes weights)
                        tt(out=fy, in0=fy, in1=v, op=Alu.mult)
                        tt(out=fy1, in0=fy1, in1=v, op=Alu.mult)

                    def c_wrow0():
                        fx, fy = hx(0), hx(1)
                        fx1, fy1, nm, t1 = hx(4), hx(5), hx(6), hx(7)
                        pm = hx(2)
                        # corner weights: wa=fx1*fy1 wb=fx1*fy wc=fx*fy1
                        # wd=fx*fy; 8-slot vector (slot = r*4 + c):
                        #   r=0: [wa*nm, wa*pm + wc*nm, wc*pm, -]
                        #   r=1: [wb*nm, wb*pm + wd*nm, wd*pm, -]
                        wa, wb, wc = hx(5), hx(4), hx(3)
                        tt(out=wc, in0=fx, in1=fy1, op=Alu.mult)
                        tt(out=wa, in0=fx1, in1=fy1, op=Alu.mult)
                        tt(out=wb, in0=fx1, in1=fy, op=Alu.mult)
                        tt(out=hx(1), in0=fx, in1=fy, op=Alu.mult)  # wd
                        tt(out=wt(0), in0=wa, in1=nm, op=Alu.mult)
                        tt(out=t1, in0=wa, in1=pm, op=Alu.mult)
                        tt(out=hx(0), in0=wc, in1=nm, op=Alu.mult)
                        tt(out=wt(1), in0=t1, in1=hx(0), op=Alu.add)
                        tt(out=wt(2), in0=wc, in1=pm, op=Alu.mult)

                    def c_wrow1():
                        pm, nm, t1 = hx(2), hx(6), hx(7)
                        wb, wd = hx(4), hx(1)
                        tt(out=wt(4), in0=wb, in1=nm, op=Alu.mult)
                        tt(out=t1, in0=wb, in1=pm, op=Alu.mult)
                        tt(out=hx(0), in0=wd, in1=nm, op=Alu.mult)
                        tt(out=wt(5), in0=t1, in1=hx(0), op=Alu.add)
                        tt(out=wt(6), in0=wd, in1=pm, op=Alu.mult)

                    def c_dup():
                        # pair-duplicate into [P, S, 8, 2] for 2x multiplies
                        src = W8t[:].rearrange("p x s -> p s x")
                        src = src.unsqueeze(3).broadcast_to([P, PH, 8, 2])
                        cp(out=W8d[:], in_=src)

                    return [c_flow, c_coords, c_corners, c_xdec, c_ydec,
                            c_rw(0), c_rw(4), c_rep, c_valid,
                            c_wprep, c_wrow0, c_wrow1, c_dup]

                def issue_gather(ti):
                    IDXG, _ = blk[ti // TPB]
                    tl = ti % TPB
                    G = gpool.tile([P, K, ES], F16, tag="G", name=f"G{ti}")
                    nc.gpsimd.dma_gather(
                        G[:], img[ti // (NT // BPC)],
                        IDXG[:, tl * (K * 8):(tl + 1) * (K * 8)],
                        P * K, P * K, ES, single_packet=False,
                    )
                    return G

                otile = {}

                def process_eng(ti, G, lo, hi, eng):
                    # one engine's pixel range end-to-end: multiplies, folds
                    # and its slice of the output. Weight slots 3 and 7 are
                    # structurally zero; skip them.
                    sl = slice(ti * K, (ti + 1) * K)
                    _, W8d = blk[ti // TPB]
                    tl = ti % TPB
                    G4 = G[:].rearrange("p k (x c) -> p k x c", c=C)
                    if ti not in otile:
                        otile[ti] = opool.tile([P, K, C], F16, tag="O",
                                               name=f"O{ti}")
                    O = otile[ti]
                    kr = slice(lo, hi)
                    wsl = slice(tl * K + lo, tl * K + hi)
                    g4 = G4[:, kr]
                    # merged (pixel, slot) dim keeps the AP at 3 free dims
                    # (the ISA limit); slots 3/7 carry zero weights
                    gm = g4.rearrange("p k x (h d) -> p (k x) h d", d=2)
                    wm = W8d[:, wsl].rearrange("p k x d -> p (k x) d")
                    wm = wm.unsqueeze(2).broadcast_to(
                        [P, (hi - lo) * 8, C // 2, 2])
                    eng.tensor_tensor(out=gm, in0=gm, in1=wm, op=Alu.mult)
                    eng.tensor_tensor(out=g4[:, :, 0:3, :],
                                      in0=g4[:, :, 0:3, :],
                                      in1=g4[:, :, 4:7, :], op=Alu.add)
                    eng.tensor_tensor(out=g4[:, :, 0, :],
                                      in0=g4[:, :, 0, :],
                                      in1=g4[:, :, 1, :], op=Alu.add)
                    eng.tensor_tensor(out=O[:, kr], in0=g4[:, :, 0, :],
                                      in1=g4[:, :, 2, :], op=Alu.add)
                    nc.sync.dma_start(out=out_v[:, sl, :][:, kr],
                                      in_=O[:, kr])

                # software pipeline: gathers run LOOKAHEAD tiles ahead of
                # processing; the NEXT block's index/weight chunks are
                # spread between this block's tile pumps so they fill the
                # idle gaps of the DMA-bound steady state.
                pending = []

                def pump(ti):
                    pending.append((ti, issue_gather(ti)))
                    if len(pending) >= LOOKAHEAD:
                        t1, G1 = pending.pop(0)
                        process_eng(t1, G1, 0, KD, nc.vector)
                        process_eng(t1, G1, KD, K, nc.gpsimd)

                for ch in phase_chunks(0):
                    ch()
                for bi in range(NPH):
                    chunks = phase_chunks(bi + 1) if bi + 1 < NPH else []
                    done = 0
                    for j in range(TPB):
                        pump(bi * TPB + j)
                        want = len(chunks) * (j + 1) // TPB
                        while done < want:
                            chunks[done]()
                            done += 1
                for ta, Ga in pending:
                    process_eng(ta, Ga, 0, 51, nc.vector)
                    process_eng(ta, Ga, 51, K, nc.gpsimd)

    nc.compile()
    return nc


_CACHED = {}


def _get_program():
    if "nc" not in _CACHED:
        _CACHED["nc"] = _build_program()
    return _CACHED["nc"]


def _linspace_f32(n):
    # match jnp.linspace(-1, 1, n, dtype=float32): iota*step + start in f32
    step = np.float32(2.0) / np.float32(n - 1)
    return (np.arange(n, dtype=np.float32) * step + np.float32(-1.0)).astype(
        np.float32)


def _host_tables():
    # pixel (p, s): per-core pixel id g = s*128 + p; within-image id
    # pid = g % IMG_PIX; j = pid % W = p + 128*(s % 2);
    # i = pid // W = (s % HS) // 2
    xs = _linspace_f32(W)
    ys = _linspace_f32(H)
    xbt = np.stack([xs[:P], xs[P:2 * P]], axis=1)          # [P, 2]
    ybt = np.broadcast_to(ys[:HS // 2], (P, HS // 2))      # [P, 256]
    return np.ascontiguousarray(xbt), np.ascontiguousarray(ybt)


def _repack_images(image16):
    # image16: [nb, H, W, C] fp16 -> per image entries
    # [par, hp, u, r, c4, ch] = Ipad[2*hp + par + r, 2*u + c, ch]
    nb = image16.shape[0]
    pad = np.zeros((nb, H + 2, W + 2, C), np.float16)
    pad[:, :H, :W, :] = image16
    sb, sh, sw, sc = pad.strides
    ev = np.lib.stride_tricks.as_strided(
        pad,
        shape=(nb, 2, H // 2, W // 2, 2, 4, C),
        strides=(sb, sh, 2 * sh, 2 * sw, sh, sw, sc),
    )
    return np.ascontiguousarray(ev).reshape(nb, NE, ES)


def kernel(image: np.ndarray, flow: np.ndarray) -> np.ndarray:
    image = np.asarray(image)
    flow = np.asarray(flow)
    assert image.shape == (B, H, W, C) and flow.shape == (B, H, W, 2)

    nc = _get_program()
    xbt, ybt = _host_tables()

    entries = _repack_images(image.astype(np.float16))  # [B, NE, ES]
    flow32 = np.ascontiguousarray(flow, dtype=np.float32).reshape(
        NCORES, SLOTS, P, 2).transpose(0, 2, 1, 3).reshape(NCORES, NPIX, 2)

    in_maps = []
    for c in range(NCORES):
        in_maps.append({
            "img": np.ascontiguousarray(entries[BPC * c:BPC * (c + 1)]),
            "flow": np.ascontiguousarray(flow32[c]),
            "xbt": xbt,
            "ybt": ybt,
        })

    res = run_bass_kernel_spmd(nc, in_maps, list(range(NCORES)))
    _CACHED["last_result"] = res
    outs = [res.results[c]["out"].reshape(P, SLOTS, C).transpose(1, 0, 2)
            .reshape(NPIX, C) for c in range(NCORES)]
    return np.concatenate(outs, axis=0).reshape(B, H, W, C).astype(np.float32)


# revision 43
# speedup vs baseline: 1.0046x; 1.0046x over previous
"""Bilinear sampler (flow warp) on 8 Trainium2 NeuronCores.

reference semantics (per batch b, f32):
  xn = 0.3*flow_x + linspace(-1,1,W)[j];  x = (xn + 1) * W/2
  yn = 0.3*flow_y + linspace(-1,1,H)[i];  y = (yn + 1) * H/2
  out = bilinear sample of image at (y, x). When x is outside [0, W-1)
  or y outside [0, H-1), the reference's clipped corner indices
  coincide and the weights cancel exactly, so out = 0 there.

Strategy: pure data parallel, 2 images per core.

The image is host-repacked into "entries" of 2 rows x 4 cols x 32 ch
(fp16, 512B): entry (par, hp, u) = rows (2*hp+par, 2*hp+par+1), cols
(2*u .. 2*u+3). For any corner (y0, x0) the entry
  idx = par*16384 + hp*128 + u,  par = y0&1, hp = y0>>1, u = x0>>1
contains the full 2x2 bilinear neighborhood; idx <= 32767 fits the
int16 index of gpsimd.dma_gather, and ONE 512B gather descriptor per
output pixel fetches all 4 corners.

Per-pixel weights are stored PAIR-DUPLICATED ([..., slot, 2] fp16) so
the broadcast multiply against the gathered tile has a packed
innermost dim on every operand and runs in the DVE 2x perf mode. Each
tile's pixel range is split between DVE and GpSimd (Pool), each engine
handling its range end-to-end (multiplies, folds, its output slice) so
neither stalls on the other. Output is written fp16 and upcast on the
host.

The index/weight computation is emitted in small chunks interleaved
between tile pumps: each chunk fills the idle gap DVE has while the
DMA-bound gather stream runs, so the next block's gather indices are
always ready just before its first gather issues.

Pixel layout: per-core pixel g = s*128 + p (p = partition). dma_gather
writes chunk i to partition i%128, so a tile of P*K pixels lands as
[128, K]; its int16 indices are consumed from [i%16, i//16], which the
kernel builds into a rewrapped index tile via DMA+DVE shuffles.
"""

import numpy as np

import concourse.bacc as bacc
import concourse.mybir as mybir
import concourse.tile as tile
from concourse import library_config
from concourse.bass_utils import run_bass_kernel_spmd

B, H, W, C = 16, 256, 256, 32
NCORES = 8
BPC = B // NCORES            # images per core
NPIX = BPC * H * W           # pixels per core (131072)
P = 128                      # partitions
SLOTS = NPIX // P            # free slots per partition (1024)
K = 64                       # pixel slots per gather tile
NT = SLOTS // K              # gather tiles per core (16)
KD = 55                      # pixel slots of each tile processed on DVE
                             # (the rest go to GpSimd/Pool)
IMG_PIX = H * W              # pixels per image (65536)
NE = 2 * (H // 2) * (W // 2)  # entries per image (32768)
ES = 2 * 4 * C               # elems per entry (256 fp16 = 512B)
HS = SLOTS // 2              # per-image slot count (512)
PH = 512                     # phase block size (slots); divides HS
NPH = SLOTS // PH            # phase blocks (4)
TPB = PH // K                # tiles per phase block (4)
LOOKAHEAD = 2                # gathers in flight ahead of processing

F32 = mybir.dt.float32
F16 = mybir.dt.float16
I32 = mybir.dt.int32
I16 = mybir.dt.int16
Alu = mybir.AluOpType


def _build_program():
    nc = bacc.Bacc("TRN2", target_bir_lowering=False, debug=False,
                   num_devices=NCORES)

    img = nc.dram_tensor("img", [BPC, NE, ES], F16, kind="ExternalInput").ap()
    flow = nc.dram_tensor("flow", [NPIX, 2], F32, kind="ExternalInput").ap()
    xbt = nc.dram_tensor("xbt", [P, 2], F32, kind="ExternalInput").ap()
    ybt = nc.dram_tensor("ybt", [P, HS // 2], F32, kind="ExternalInput").ap()
    out = nc.dram_tensor("out", [NPIX, C], F16, kind="ExternalOutput").ap()

    # flow/out are host-permuted to (p, s) order so these DMAs move
    # contiguous per-partition runs instead of tiny scatters
    out_v = out.rearrange("(p s) c -> p s c", p=P)
    flow_v = flow.rearrange("(p s) c -> p s c", p=P)

    with tile.TileContext(nc) as tc:
        with (
            tc.tile_pool(name="gather", bufs=3) as gpool,
            tc.tile_pool(name="outp", bufs=2) as opool,
            tc.tile_pool(name="idxp", bufs=2) as ipool,
            tc.tile_pool(name="wp", bufs=2) as wpool,
        ):
            nc.gpsimd.load_library(library_config.mlp)
            # per-phase-block ring tiles: gather indices and duplicated
            # weights live only from their phase block to its last tile
            blk = {}

            with (
                tc.tile_pool(name="tmp", bufs=1) as tpool,
                tc.tile_pool(name="tmp2", bufs=1) as t2pool,
            ):
                XB = t2pool.tile([P, 2], F32, tag="XB")
                YB = t2pool.tile([P, HS // 2], F32, tag="YB")
                IDXC = t2pool.tile([P, PH], I16, tag="IDXC")
                SC = t2pool.tile([16, 8, PH], I16, tag="SC")
                II = t2pool.tile([P, PH], I32, tag="II")
                # fp16 intermediates for the weight phase, slot-major so
                # every op reads/writes a packed innermost dim
                H16 = t2pool.tile([P, 8, PH], F16, tag="H16")
                W8t = t2pool.tile([P, 8, PH], F16, tag="W8t")
                nc.vector.memzero(W8t[:, 3:8:4, :])

                ts, tt, cp = (nc.vector.tensor_scalar,
                              nc.vector.tensor_tensor,
                              nc.vector.tensor_copy)

                def t(tag):
                    return tpool.tile([P, PH], F32, tag=tag, name=tag)[:]

                def phase_chunks(bi):
                    """Index+weight computation for phase block bi as a list
                    of small emission chunks (each ~1-3 us of DVE time)."""
                    s0 = bi * PH
                    F = t2pool.tile([P, PH, 2], F32, tag="F", name=f"F{bi}")
                    IDXG = ipool.tile([P, PH * 8], I16, tag="IDXG",
                                      name=f"IDXG{bi}")
                    W8d = wpool.tile([P, PH, 8, 2], F16, tag="W8d",
                                     name=f"W8d{bi}")
                    blk[bi] = (IDXG, W8d)
                    sl = slice(0, PH)

                    def c_flow():
                        nc.sync.dma_start(
                            out=F[:], in_=flow_v[:, s0:s0 + PH, :])
                    xp, yp = t("xp"), t("yp")
                    v, vt = t("v"), t("vt")
                    cx, cy = t("cx"), t("cy")
                    # tag-aliased successors (same buffers, fresh handles);
                    # hoisted so the chunk that writes and the chunk that
                    # reads share one tile object
                    uf, pmv = t("uf"), t("v")
                    hpf, pr, idf = t("hpf"), t("cx"), t("cy")
                    ii = II[:]

                    def c_coords():
                        # xb alternates xs[p], xs[128+p] along s; yb steps
                        # every other s; broadcast views over s-pairs
                        xb = XB[:].unsqueeze(1).broadcast_to([P, PH // 2, 2])
                        i0 = (s0 % HS) // 2
                        ybq = YB[:, i0:i0 + PH // 2]
                        yb = ybq.unsqueeze(2).broadcast_to([P, PH // 2, 2])
                        xp2 = xp.rearrange("p (a b) -> p a b", b=2)
                        yp2 = yp.rearrange("p (a b) -> p a b", b=2)
                        ts(out=xp, in0=F[:, :, 0], scalar1=0.3, scalar2=None,
                           op0=Alu.mult)
                        tt(out=xp2, in0=xp2, in1=xb, op=Alu.add)
                        ts(out=xp, in0=xp, scalar1=1.0,
                           scalar2=float(W) / 2.0, op0=Alu.add, op1=Alu.mult)
                        ts(out=yp, in0=F[:, :, 1], scalar1=0.3, scalar2=None,
                           op0=Alu.mult)
                        tt(out=yp2, in0=yp2, in1=yb, op=Alu.add)
                        ts(out=yp, in0=yp, scalar1=1.0,
                           scalar2=float(H) / 2.0, op0=Alu.add, op1=Alu.mult)

                    def c_valid():
                        # 0 <= x < W-1 and 0 <= y < H-1 (else out = 0);
                        # result lands in fp16 lane 3 of H16
                        ts(out=v, in0=xp, scalar1=0.0, scalar2=None,
                           op0=Alu.is_ge)
                        ts(out=vt, in0=xp, scalar1=float(W - 1), scalar2=None,
                           op0=Alu.is_lt)
                        tt(out=v, in0=v, in1=vt, op=Alu.mult)
                        ts(out=vt, in0=yp, scalar1=0.0, scalar2=None,
                           op0=Alu.is_ge)
                        tt(out=v, in0=v, in1=vt, op=Alu.mult)
                        ts(out=vt, in0=yp, scalar1=float(H - 1), scalar2=None,
                           op0=Alu.is_lt)
                        tt(out=H16[:, 3, sl], in0=v, in1=vt, op=Alu.mult)

                    def c_corners():
                        # round(x - 0.5) == floor(x) away from halves,
                        # matching the reference; clamp to [0, W-2]
                        xh = t("vt")
                        ts(out=xh, in0=xp, scalar1=-0.5, scalar2=None,
                           op0=Alu.add)
                        cp(out=ii, in_=xh)
                        cp(out=cx, in_=ii)
                        ts(out=cx, in0=cx, scalar1=0.0, scalar2=float(W - 2),
                           op0=Alu.max, op1=Alu.min)
                        yh = t("vt")
                        ts(out=yh, in0=yp, scalar1=-0.5, scalar2=None,
                           op0=Alu.add)
                        cp(out=ii, in_=yh)
                        cp(out=cy, in_=ii)
                        ts(out=cy, in0=cy, scalar1=0.0, scalar2=float(H - 2),
                           op0=Alu.max, op1=Alu.min)

                    def c_xdec():
                        # fractions -> fp16 lanes; u = x0>>1, pm = x0&1
                        tt(out=H16[:, 0, sl], in0=xp, in1=cx, op=Alu.subtract)
                        tt(out=H16[:, 1, sl], in0=yp, in1=cy, op=Alu.subtract)
                        ts(out=uf, in0=cx, scalar1=0.5, scalar2=-0.25,
                           op0=Alu.mult, op1=Alu.add)
                        cp(out=ii, in_=uf)
                        cp(out=uf, in_=ii)
                        ts(out=pmv, in0=uf, scalar1=-2.0, scalar2=None,
                           op0=Alu.mult)
                        tt(out=pmv, in0=pmv, in1=cx, op=Alu.add)
                        cp(out=H16[:, 2, sl], in_=pmv)

                    def c_ydec():
                        ts(out=hpf, in0=cy, scalar1=0.5, scalar2=-0.25,
                           op0=Alu.mult, op1=Alu.add)
                        cp(out=ii, in_=hpf)
                        cp(out=hpf, in_=ii)
                        ts(out=pr, in0=hpf, scalar1=-2.0, scalar2=None,
                           op0=Alu.mult)
                        tt(out=pr, in0=pr, in1=cy, op=Alu.add)
                        # entry index = pr*16384 + hpf*128 + u (int16-exact)
                        ts(out=idf, in0=pr, scalar1=float(NE // 2),
                           scalar2=None, op0=Alu.mult)
                        ts(out=hpf, in0=hpf, scalar1=float(W // 2),
                           scalar2=None, op0=Alu.mult)
                        tt(out=idf, in0=idf, in1=hpf, op=Alu.add)
                        tt(out=idf, in0=idf, in1=uf, op=Alu.add)
                        cp(out=IDXC[:, sl], in_=idf)

                    # rewrap indices for dma_gather: chunk i reads its int16
                    # index from [i%16, i//16] (replicated per 16-partition
                    # group); i = s_local*128 + p  =>
                    #   IDXG[16*rep + p%16, s*8 + p//16] = IDXC[p, s]
                    cols = slice(0, PH * 8)
                    dst = IDXG[0:16, cols].rearrange("p (s g) -> p s g", g=8)

                    def c_rw(g0):
                        def f():
                            for g in range(g0, g0 + 4):
                                nc.sync.dma_start(
                                    out=SC[:, g, :],
                                    in_=IDXC[16 * g:16 * (g + 1), :])
                                nc.vector.tensor_copy(out=dst[:, :, g],
                                                      in_=SC[:, g, :])
                        return f

                    def c_rep():
                        for rep in range(1, 8):
                            nc.sync.dma_start(
                                out=IDXG[16 * rep:16 * (rep + 1), cols],
                                in_=IDXG[0:16, cols])

                    def hx(i):
                        return H16[:, i, sl]

                    def wt(i):
                        return W8t[:, i, sl]

                    def c_wprep():
                        fx, fy, pm, v = hx(0), hx(1), hx(2), hx(3)
                        fx1, fy1, nm = hx(4), hx(5), hx(6)
                        ts(out=fx1, in0=fx, scalar1=-1.0, scalar2=1.0,
                           op0=Alu.mult, op1=Alu.add)
                        ts(out=fy1, in0=fy, scalar1=-1.0, scalar2=1.0,
                           op0=Alu.mult, op1=Alu.add)
                        ts(out=nm, in0=pm, scalar1=-1.0, scalar2=1.0,
                           op0=Alu.mult, op1=Alu.add)
                        # fold validity into the y factors (zeroes weights)
                        tt(out=fy, in0=fy, in1=v, op=Alu.mult)
                        tt(out=fy1, in0=fy1, in1=v, op=Alu.mult)

                    def c_wrow0():
                        fx, fy = hx(0), hx(1)
                        fx1, fy1, nm, t1 = hx(4), hx(5), hx(6), hx(7)
                        pm = hx(2)
                        # corner weights: wa=fx1*fy1 wb=fx1*fy wc=fx*fy1
                        # wd=fx*fy; 8-slot vector (slot = r*4 + c):
                        #   r=0: [wa*nm, wa*pm + wc*nm, wc*pm, -]
                        #   r=1: [wb*nm, wb*pm + wd*nm, wd*pm, -]
                        wa, wb, wc = hx(5), hx(4), hx(3)
                        tt(out=wc, in0=fx, in1=fy1, op=Alu.mult)
                        tt(out=wa, in0=fx1, in1=fy1, op=Alu.mult)
                        tt(out=wb, in0=fx1, in1=fy, op=Alu.mult)
                        tt(out=hx(1), in0=fx, in1=fy, op=Alu.mult)  # wd
                        tt(out=wt(0), in0=wa, in1=nm, op=Alu.mult)
                        tt(out=t1, in0=wa, in1=pm, op=Alu.mult)
                        tt(out=hx(0), in0=wc, in1=nm, op=Alu.mult)
                        tt(out=wt(1), in0=t1, in1=hx(0), op=Alu.add)
                        tt(out=wt(2), in0=wc, in1=pm, op=Alu.mult)

                    def c_wrow1():
                        pm, nm, t1 = hx(2), hx(6), hx(7)
                        wb, wd = hx(4), hx(1)
                        tt(out=wt(4), in0=wb, in1=nm, op=Alu.mult)
                        tt(out=t1, in0=wb, in1=pm, op=Alu.mult)
                        tt(out=hx(0), in0=wd, in1=nm, op=Alu.mult)
                        tt(out=wt(5), in0=t1, in1=hx(0), op=Alu.add)
                        tt(out=wt(6), in0=wd, in1=pm, op=Alu.mult)

                    def c_dup():
                        # pair-duplicate into [P, S, 8, 2] for 2x multiplies
                        src = W8t[:].rearrange("p x s -> p s x")
                        src = src.unsqueeze(3).broadcast_to([P, PH, 8, 2])
                        cp(out=W8d[:], in_=src)

                    return [c_flow, c_coords, c_corners, c_xdec, c_ydec,
                            c_rw(0), c_rw(4), c_rep, c_valid,
                            c_wprep, c_wrow0, c_wrow1, c_dup]

                def issue_gather(ti):
                    IDXG, _ = blk[ti // TPB]
                    tl = ti % TPB
                    G = gpool.tile([P, K, ES], F16, tag="G", name=f"G{ti}")
                    nc.gpsimd.dma_gather(
                        G[:], img[ti // (NT // BPC)],
                        IDXG[:, tl * (K * 8):(tl + 1) * (K * 8)],
                        P * K, P * K, ES, single_packet=False,
                    )
                    return G

                otile = {}

                def process_eng(ti, G, lo, hi, eng):
                    # one engine's pixel range end-to-end: multiplies, folds
                    # and its slice of the output. Weight slots 3 and 7 are
                    # structurally zero; skip them.
                    sl = slice(ti * K, (ti + 1) * K)
                    _, W8d = blk[ti // TPB]
                    tl = ti % TPB
                    G4 = G[:].rearrange("p k (x c) -> p k x c", c=C)
                    if ti not in otile:
                        otile[ti] = opool.tile([P, K, C], F16, tag="O",
                                               name=f"O{ti}")
                    O = otile[ti]
                    kr = slice(lo, hi)
                    wsl = slice(tl * K + lo, tl * K + hi)
                    g4 = G4[:, kr]
                    # merged (pixel, slot) dim keeps the AP at 3 free dims
                    # (the ISA limit); slots 3/7 carry zero weights
                    gm = g4.rearrange("p k x (h d) -> p (k x) h d", d=2)
                    wm = W8d[:, wsl].rearrange("p k x d -> p (k x) d")
                    wm = wm.unsqueeze(2).broadcast_to(
                        [P, (hi - lo) * 8, C // 2, 2])
                    eng.tensor_tensor(out=gm, in0=gm, in1=wm, op=Alu.mult)
                    eng.tensor_tensor(out=g4[:, :, 0:3, :],
                                      in0=g4[:, :, 0:3, :],
                                      in1=g4[:, :, 4:7, :], op=Alu.add)
                    eng.tensor_tensor(out=g4[:, :, 0, :],
                                      in0=g4[:, :, 0, :],
                                      in1=g4[:, :, 1, :], op=Alu.add)
                    eng.tensor_tensor(out=O[:, kr], in0=g4[:, :, 0, :],
                                      in1=g4[:, :, 2, :], op=Alu.add)
                    nc.sync.dma_start(out=out_v[:, sl, :][:, kr],
                                      in_=O[:, kr])

                # software pipeline: gathers run LOOKAHEAD tiles ahead of
                # processing; the NEXT block's index/weight chunks are
                # spread between this block's tile pumps so they fill the
                # idle gaps of the DMA-bound steady state.
                pending = []

                def pump(ti):
                    pending.append((ti, issue_gather(ti)))
                    if len(pending) >= LOOKAHEAD:
                        t1, G1 = pending.pop(0)
                        process_eng(t1, G1, 0, KD, nc.vector)
                        process_eng(t1, G1, KD, K, nc.gpsimd)

                ch0 = phase_chunks(0)
                ch0[0]()  # flow load for block 0 starts first
                nc.sync.dma_start(out=XB[:], in_=xbt[:])
                nc.sync.dma_start(out=YB[:], in_=ybt[:])
                for ch in ch0[1:]:
                    ch()
                for bi in range(NPH):
                    chunks = phase_chunks(bi + 1) if bi + 1 < NPH else []
                    done = 0
                    for j in range(TPB):
                        pump(bi * TPB + j)
                        want = len(chunks) * (j + 1) // TPB
                        while done < want:
                            chunks[done]()
                            done += 1
                for ta, Ga in pending:
                    process_eng(ta, Ga, 0, 51, nc.vector)
                    process_eng(ta, Ga, 51, K, nc.gpsimd)

    nc.compile()
    return nc


_CACHED = {}


def _get_program():
    if "nc" not in _CACHED:
        _CACHED["nc"] = _build_program()
    return _CACHED["nc"]


def _linspace_f32(n):
    # match jnp.linspace(-1, 1, n, dtype=float32): iota*step + start in f32
    step = np.float32(2.0) / np.float32(n - 1)
    return (np.arange(n, dtype=np.float32) * step + np.float32(-1.0)).astype(
        np.float32)


def _host_tables():
    # pixel (p, s): per-core pixel id g = s*128 + p; within-image id
    # pid = g % IMG_PIX; j = pid % W = p + 128*(s % 2);
    # i = pid // W = (s % HS) // 2
    xs = _linspace_f32(W)
    ys = _linspace_f32(H)
    xbt = np.stack([xs[:P], xs[P:2 * P]], axis=1)          # [P, 2]
    ybt = np.broadcast_to(ys[:HS // 2], (P, HS // 2))      # [P, 256]
    return np.ascontiguousarray(xbt), np.ascontiguousarray(ybt)


def _repack_images(image16):
    # image16: [nb, H, W, C] fp16 -> per image entries
    # [par, hp, u, r, c4, ch] = Ipad[2*hp + par + r, 2*u + c, ch]
    nb = image16.shape[0]
    pad = np.zeros((nb, H + 2, W + 2, C), np.float16)
    pad[:, :H, :W, :] = image16
    sb, sh, sw, sc = pad.strides
    ev = np.lib.stride_tricks.as_strided(
        pad,
        shape=(nb, 2, H // 2, W // 2, 2, 4, C),
        strides=(sb, sh, 2 * sh, 2 * sw, sh, sw, sc),
    )
    return np.ascontiguousarray(ev).reshape(nb, NE, ES)


def kernel(image: np.ndarray, flow: np.ndarray) -> np.ndarray:
    image = np.asarray(image)
    flow = np.asarray(flow)
    assert image.shape == (B, H, W, C) and flow.shape == (B, H, W, 2)

    nc = _get_program()
    xbt, ybt = _host_tables()

    entries = _repack_images(image.astype(np.float16))  # [B, NE, ES]
    flow32 = np.ascontiguousarray(flow, dtype=np.float32).reshape(
        NCORES, SLOTS, P, 2).transpose(0, 2, 1, 3).reshape(NCORES, NPIX, 2)

    in_maps = []
    for c in range(NCORES):
        in_maps.append({
            "img": np.ascontiguousarray(entries[BPC * c:BPC * (c + 1)]),
            "flow": np.ascontiguousarray(flow32[c]),
            "xbt": xbt,
            "ybt": ybt,
        })

    res = run_bass_kernel_spmd(nc, in_maps, list(range(NCORES)))
    _CACHED["last_result"] = res
    outs = [res.results[c]["out"].reshape(P, SLOTS, C).transpose(1, 0, 2)
            .reshape(NPIX, C) for c in range(NCORES)]
    return np.concatenate(outs, axis=0).reshape(B, H, W, C).astype(np.float32)


# revision 44
# speedup vs baseline: 1.0092x; 1.0046x over previous
"""Bilinear sampler (flow warp) on 8 Trainium2 NeuronCores.

reference semantics (per batch b, f32):
  xn = 0.3*flow_x + linspace(-1,1,W)[j];  x = (xn + 1) * W/2
  yn = 0.3*flow_y + linspace(-1,1,H)[i];  y = (yn + 1) * H/2
  out = bilinear sample of image at (y, x). When x is outside [0, W-1)
  or y outside [0, H-1), the reference's clipped corner indices
  coincide and the weights cancel exactly, so out = 0 there.

Strategy: pure data parallel, 2 images per core.

The image is host-repacked into "entries" of 2 rows x 4 cols x 32 ch
(fp16, 512B): entry (par, hp, u) = rows (2*hp+par, 2*hp+par+1), cols
(2*u .. 2*u+3). For any corner (y0, x0) the entry
  idx = par*16384 + hp*128 + u,  par = y0&1, hp = y0>>1, u = x0>>1
contains the full 2x2 bilinear neighborhood; idx <= 32767 fits the
int16 index of gpsimd.dma_gather, and ONE 512B gather descriptor per
output pixel fetches all 4 corners.

Per-pixel weights are stored PAIR-DUPLICATED ([..., slot, 2] fp16) so
the broadcast multiply against the gathered tile has a packed
innermost dim on every operand and runs in the DVE 2x perf mode. Each
tile's pixel range is split between DVE and GpSimd (Pool), each engine
handling its range end-to-end (multiplies, folds, its output slice) so
neither stalls on the other. Output is written fp16 and upcast on the
host.

The index/weight computation is emitted in small chunks interleaved
between tile pumps: each chunk fills the idle gap DVE has while the
DMA-bound gather stream runs, so the next block's gather indices are
always ready just before its first gather issues.

Pixel layout: per-core pixel g = s*128 + p (p = partition). dma_gather
writes chunk i to partition i%128, so a tile of P*K pixels lands as
[128, K]; its int16 indices are consumed from [i%16, i//16], which the
kernel builds into a rewrapped index tile via DMA+DVE shuffles.
"""

import numpy as np

import concourse.bacc as bacc
import concourse.mybir as mybir
import concourse.tile as tile
from concourse import library_config
from concourse.bass_utils import run_bass_kernel_spmd

B, H, W, C = 16, 256, 256, 32
NCORES = 8
BPC = B // NCORES            # images per core
NPIX = BPC * H * W           # pixels per core (131072)
P = 128                      # partitions
SLOTS = NPIX // P            # free slots per partition (1024)
K = 64                       # pixel slots per gather tile
NT = SLOTS // K              # gather tiles per core (16)
KD = 55                      # pixel slots of each tile processed on DVE
                             # (the rest go to GpSimd/Pool)
IMG_PIX = H * W              # pixels per image (65536)
NE = 2 * (H // 2) * (W // 2)  # entries per image (32768)
ES = 2 * 4 * C               # elems per entry (256 fp16 = 512B)
HS = SLOTS // 2              # per-image slot count (512)
PH = 512                     # phase block size (slots); divides HS
NPH = SLOTS // PH            # phase blocks (4)
TPB = PH // K                # tiles per phase block (4)
LOOKAHEAD = 2                # gathers in flight ahead of processing

F32 = mybir.dt.float32
F16 = mybir.dt.float16
I32 = mybir.dt.int32
I16 = mybir.dt.int16
Alu = mybir.AluOpType


def _build_program():
    nc = bacc.Bacc("TRN2", target_bir_lowering=False, debug=False,
                   num_devices=NCORES)

    img = nc.dram_tensor("img", [BPC, NE, ES], F16, kind="ExternalInput").ap()
    flow = nc.dram_tensor("flow", [NPIX, 2], F32, kind="ExternalInput").ap()
    xbt = nc.dram_tensor("xbt", [P, 2], F32, kind="ExternalInput").ap()
    ybt = nc.dram_tensor("ybt", [P, HS // 2], F32, kind="ExternalInput").ap()
    out = nc.dram_tensor("out", [NPIX, C], F16, kind="ExternalOutput").ap()

    # flow/out are host-permuted to (p, s) order so these DMAs move
    # contiguous per-partition runs instead of tiny scatters
    out_v = out.rearrange("(p s) c -> p s c", p=P)
    flow_v = flow.rearrange("(p s) c -> p s c", p=P)

    with tile.TileContext(nc) as tc:
        with (
            tc.tile_pool(name="gather", bufs=3) as gpool,
            tc.tile_pool(name="outp", bufs=2) as opool,
            tc.tile_pool(name="idxp", bufs=2) as ipool,
            tc.tile_pool(name="wp", bufs=2) as wpool,
        ):
            nc.gpsimd.load_library(library_config.mlp)
            # per-phase-block ring tiles: gather indices and duplicated
            # weights live only from their phase block to its last tile
            blk = {}

            with (
                tc.tile_pool(name="tmp", bufs=1) as tpool,
                tc.tile_pool(name="tmp2", bufs=1) as t2pool,
            ):
                XB = t2pool.tile([P, 2], F32, tag="XB")
                YB = t2pool.tile([P, HS // 2], F32, tag="YB")
                IDXC = t2pool.tile([P, PH], I16, tag="IDXC")
                SC = t2pool.tile([16, 8, PH], I16, tag="SC")
                II = t2pool.tile([P, PH], I32, tag="II")
                # fp16 intermediates for the weight phase, slot-major so
                # every op reads/writes a packed innermost dim
                H16 = t2pool.tile([P, 8, PH], F16, tag="H16")
                W8t = t2pool.tile([P, 8, PH], F16, tag="W8t")
                nc.vector.memzero(W8t[:, 3:8:4, :])

                ts, tt, cp = (nc.vector.tensor_scalar,
                              nc.vector.tensor_tensor,
                              nc.vector.tensor_copy)

                def t(tag):
                    return tpool.tile([P, PH], F32, tag=tag, name=tag)[:]

                def phase_chunks(bi):
                    """Index+weight computation for phase block bi as a list
                    of small emission chunks (each ~1-3 us of DVE time)."""
                    s0 = bi * PH
                    F = t2pool.tile([P, PH, 2], F32, tag="F", name=f"F{bi}")
                    IDXG = ipool.tile([P, PH * 8], I16, tag="IDXG",
                                      name=f"IDXG{bi}")
                    W8d = wpool.tile([P, PH, 8, 2], F16, tag="W8d",
                                     name=f"W8d{bi}")
                    blk[bi] = (IDXG, W8d)
                    sl = slice(0, PH)

                    def c_flow():
                        nc.sync.dma_start(
                            out=F[:], in_=flow_v[:, s0:s0 + PH, :])
                    xp, yp = t("xp"), t("yp")
                    v, vt = t("v"), t("vt")
                    cx, cy = t("cx"), t("cy")
                    # tag-aliased successors (same buffers, fresh handles);
                    # hoisted so the chunk that writes and the chunk that
                    # reads share one tile object
                    uf, pmv = t("uf"), t("v")
                    hpf, pr, idf = t("hpf"), t("cx"), t("cy")
                    ii = II[:]

                    def c_coords():
                        # xb alternates xs[p], xs[128+p] along s; yb steps
                        # every other s; broadcast views over s-pairs
                        xb = XB[:].unsqueeze(1).broadcast_to([P, PH // 2, 2])
                        i0 = (s0 % HS) // 2
                        ybq = YB[:, i0:i0 + PH // 2]
                        yb = ybq.unsqueeze(2).broadcast_to([P, PH // 2, 2])
                        xp2 = xp.rearrange("p (a b) -> p a b", b=2)
                        yp2 = yp.rearrange("p (a b) -> p a b", b=2)
                        ts(out=xp, in0=F[:, :, 0], scalar1=0.3, scalar2=None,
                           op0=Alu.mult)
                        tt(out=xp2, in0=xp2, in1=xb, op=Alu.add)
                        ts(out=xp, in0=xp, scalar1=1.0,
                           scalar2=float(W) / 2.0, op0=Alu.add, op1=Alu.mult)
                        ts(out=yp, in0=F[:, :, 1], scalar1=0.3, scalar2=None,
                           op0=Alu.mult)
                        tt(out=yp2, in0=yp2, in1=yb, op=Alu.add)
                        ts(out=yp, in0=yp, scalar1=1.0,
                           scalar2=float(H) / 2.0, op0=Alu.add, op1=Alu.mult)

                    def c_valid():
                        # 0 <= x < W-1 and 0 <= y < H-1 (else out = 0);
                        # result lands in fp16 lane 3 of H16
                        ts(out=v, in0=xp, scalar1=0.0, scalar2=None,
                           op0=Alu.is_ge)
                        ts(out=vt, in0=xp, scalar1=float(W - 1), scalar2=None,
                           op0=Alu.is_lt)
                        tt(out=v, in0=v, in1=vt, op=Alu.mult)
                        ts(out=vt, in0=yp, scalar1=0.0, scalar2=None,
                           op0=Alu.is_ge)
                        tt(out=v, in0=v, in1=vt, op=Alu.mult)
                        ts(out=vt, in0=yp, scalar1=float(H - 1), scalar2=None,
                           op0=Alu.is_lt)
                        tt(out=H16[:, 3, sl], in0=v, in1=vt, op=Alu.mult)

                    def c_corners():
                        # round(x - 0.5) == floor(x) away from halves,
                        # matching the reference; clamp to [0, W-2]
                        xh = t("vt")
                        ts(out=xh, in0=xp, scalar1=-0.5, scalar2=None,
                           op0=Alu.add)
                        cp(out=ii, in_=xh)
                        cp(out=cx, in_=ii)
                        ts(out=cx, in0=cx, scalar1=0.0, scalar2=float(W - 2),
                           op0=Alu.max, op1=Alu.min)
                        yh = t("vt")
                        ts(out=yh, in0=yp, scalar1=-0.5, scalar2=None,
                           op0=Alu.add)
                        cp(out=ii, in_=yh)
                        cp(out=cy, in_=ii)
                        ts(out=cy, in0=cy, scalar1=0.0, scalar2=float(H - 2),
                           op0=Alu.max, op1=Alu.min)

                    def c_xdec():
                        # fractions -> fp16 lanes; u = x0>>1, pm = x0&1
                        tt(out=H16[:, 0, sl], in0=xp, in1=cx, op=Alu.subtract)
                        tt(out=H16[:, 1, sl], in0=yp, in1=cy, op=Alu.subtract)
                        ts(out=uf, in0=cx, scalar1=0.5, scalar2=-0.25,
                           op0=Alu.mult, op1=Alu.add)
                        cp(out=ii, in_=uf)
                        cp(out=uf, in_=ii)
                        ts(out=pmv, in0=uf, scalar1=-2.0, scalar2=None,
                           op0=Alu.mult)
                        tt(out=pmv, in0=pmv, in1=cx, op=Alu.add)
                        cp(out=H16[:, 2, sl], in_=pmv)

                    def c_ydec():
                        ts(out=hpf, in0=cy, scalar1=0.5, scalar2=-0.25,
                           op0=Alu.mult, op1=Alu.add)
                        cp(out=ii, in_=hpf)
                        cp(out=hpf, in_=ii)
                        ts(out=pr, in0=hpf, scalar1=-2.0, scalar2=None,
                           op0=Alu.mult)
                        tt(out=pr, in0=pr, in1=cy, op=Alu.add)
                        # entry index = pr*16384 + hpf*128 + u (int16-exact)
                        ts(out=idf, in0=pr, scalar1=float(NE // 2),
                           scalar2=None, op0=Alu.mult)
                        ts(out=hpf, in0=hpf, scalar1=float(W // 2),
                           scalar2=None, op0=Alu.mult)
                        tt(out=idf, in0=idf, in1=hpf, op=Alu.add)
                        tt(out=idf, in0=idf, in1=uf, op=Alu.add)
                        cp(out=IDXC[:, sl], in_=idf)

                    # rewrap indices for dma_gather: chunk i reads its int16
                    # index from [i%16, i//16] (replicated per 16-partition
                    # group); i = s_local*128 + p  =>
                    #   IDXG[16*rep + p%16, s*8 + p//16] = IDXC[p, s]
                    cols = slice(0, PH * 8)
                    dst = IDXG[0:16, cols].rearrange("p (s g) -> p s g", g=8)

                    def c_rw(g0):
                        def f():
                            for g in range(g0, g0 + 4):
                                nc.sync.dma_start(
                                    out=SC[:, g, :],
                                    in_=IDXC[16 * g:16 * (g + 1), :])
                                nc.vector.tensor_copy(out=dst[:, :, g],
                                                      in_=SC[:, g, :])
                        return f

                    def c_rep():
                        for rep in range(1, 8):
                            nc.sync.dma_start(
                                out=IDXG[16 * rep:16 * (rep + 1), cols],
                                in_=IDXG[0:16, cols])

                    def hx(i):
                        return H16[:, i, sl]

                    def wt(i):
                        return W8t[:, i, sl]

                    def c_wprep():
                        fx, fy, pm, v = hx(0), hx(1), hx(2), hx(3)
                        fx1, fy1, nm = hx(4), hx(5), hx(6)
                        ts(out=fx1, in0=fx, scalar1=-1.0, scalar2=1.0,
                           op0=Alu.mult, op1=Alu.add)
                        ts(out=fy1, in0=fy, scalar1=-1.0, scalar2=1.0,
                           op0=Alu.mult, op1=Alu.add)
                        ts(out=nm, in0=pm, scalar1=-1.0, scalar2=1.0,
                           op0=Alu.mult, op1=Alu.add)
                        # fold validity into the y factors (zeroes weights)
                        tt(out=fy, in0=fy, in1=v, op=Alu.mult)
                        tt(out=fy1, in0=fy1, in1=v, op=Alu.mult)

                    def c_wrow0():
                        fx, fy = hx(0), hx(1)
                        fx1, fy1, nm, t1 = hx(4), hx(5), hx(6), hx(7)
                        pm = hx(2)
                        # corner weights: wa=fx1*fy1 wb=fx1*fy wc=fx*fy1
                        # wd=fx*fy; 8-slot vector (slot = r*4 + c):
                        #   r=0: [wa*nm, wa*pm + wc*nm, wc*pm, -]
                        #   r=1: [wb*nm, wb*pm + wd*nm, wd*pm, -]
                        wa, wb, wc = hx(5), hx(4), hx(3)
                        tt(out=wc, in0=fx, in1=fy1, op=Alu.mult)
                        tt(out=wa, in0=fx1, in1=fy1, op=Alu.mult)
                        tt(out=wb, in0=fx1, in1=fy, op=Alu.mult)
                        tt(out=hx(1), in0=fx, in1=fy, op=Alu.mult)  # wd
                        tt(out=wt(0), in0=wa, in1=nm, op=Alu.mult)
                        tt(out=t1, in0=wa, in1=pm, op=Alu.mult)
                        tt(out=hx(0), in0=wc, in1=nm, op=Alu.mult)
                        tt(out=wt(1), in0=t1, in1=hx(0), op=Alu.add)
                        tt(out=wt(2), in0=wc, in1=pm, op=Alu.mult)

                    def c_wrow1():
                        pm, nm, t1 = hx(2), hx(6), hx(7)
                        wb, wd = hx(4), hx(1)
                        tt(out=wt(4), in0=wb, in1=nm, op=Alu.mult)
                        tt(out=t1, in0=wb, in1=pm, op=Alu.mult)
                        tt(out=hx(0), in0=wd, in1=nm, op=Alu.mult)
                        tt(out=wt(5), in0=t1, in1=hx(0), op=Alu.add)
                        tt(out=wt(6), in0=wd, in1=pm, op=Alu.mult)

                    def c_dup():
                        # pair-duplicate into [P, S, 8, 2] for 2x multiplies
                        src = W8t[:].rearrange("p x s -> p s x")
                        src = src.unsqueeze(3).broadcast_to([P, PH, 8, 2])
                        cp(out=W8d[:], in_=src)

                    return [c_flow, c_coords, c_corners, c_xdec, c_ydec,
                            c_rw(0), c_rw(4), c_rep, c_valid,
                            c_wprep, c_wrow0, c_wrow1, c_dup]

                def issue_gather(ti):
                    IDXG, _ = blk[ti // TPB]
                    tl = ti % TPB
                    G = gpool.tile([P, K, ES], F16, tag="G", name=f"G{ti}")
                    nc.gpsimd.dma_gather(
                        G[:], img[ti // (NT // BPC)],
                        IDXG[:, tl * (K * 8):(tl + 1) * (K * 8)],
                        P * K, P * K, ES, single_packet=False,
                    )
                    return G

                otile = {}

                def process_eng(ti, G, lo, hi, eng):
                    # one engine's pixel range end-to-end: multiplies, folds
                    # and its slice of the output. Weight slots 3 and 7 are
                    # structurally zero; skip them.
                    sl = slice(ti * K, (ti + 1) * K)
                    _, W8d = blk[ti // TPB]
                    tl = ti % TPB
                    G4 = G[:].rearrange("p k (x c) -> p k x c", c=C)
                    if ti not in otile:
                        otile[ti] = opool.tile([P, K, C], F16, tag="O",
                                               name=f"O{ti}")
                    O = otile[ti]
                    kr = slice(lo, hi)
                    wsl = slice(tl * K + lo, tl * K + hi)
                    g4 = G4[:, kr]
                    # merged (pixel, slot) dim keeps the AP at 3 free dims
                    # (the ISA limit); slots 3/7 carry zero weights
                    gm = g4.rearrange("p k x (h d) -> p (k x) h d", d=2)
                    wm = W8d[:, wsl].rearrange("p k x d -> p (k x) d")
                    wm = wm.unsqueeze(2).broadcast_to(
                        [P, (hi - lo) * 8, C // 2, 2])
                    eng.tensor_tensor(out=gm, in0=gm, in1=wm, op=Alu.mult)
                    eng.tensor_tensor(out=g4[:, :, 0:3, :],
                                      in0=g4[:, :, 0:3, :],
                                      in1=g4[:, :, 4:7, :], op=Alu.add)
                    eng.tensor_tensor(out=g4[:, :, 0, :],
                                      in0=g4[:, :, 0, :],
                                      in1=g4[:, :, 1, :], op=Alu.add)
                    eng.tensor_tensor(out=O[:, kr], in0=g4[:, :, 0, :],
                                      in1=g4[:, :, 2, :], op=Alu.add)
                    nc.sync.dma_start(out=out_v[:, sl, :][:, kr],
                                      in_=O[:, kr])

                # software pipeline: gathers run LOOKAHEAD tiles ahead of
                # processing; the NEXT block's index/weight chunks are
                # spread between this block's tile pumps so they fill the
                # idle gaps of the DMA-bound steady state.
                pending = []

                def pump(ti):
                    pending.append((ti, issue_gather(ti)))
                    if len(pending) >= LOOKAHEAD:
                        t1, G1 = pending.pop(0)
                        process_eng(t1, G1, 0, KD, nc.vector)
                        process_eng(t1, G1, KD, K, nc.gpsimd)

                ch0 = phase_chunks(0)
                ch0[0]()  # flow load for block 0 starts first
                nc.sync.dma_start(out=XB[:], in_=xbt[:])
                nc.sync.dma_start(out=YB[:], in_=ybt[:])
                for ch in ch0[1:]:
                    ch()
                for bi in range(NPH):
                    chunks = phase_chunks(bi + 1) if bi + 1 < NPH else []
                    done = 0
                    for j in range(TPB):
                        if bi * TPB + j < NT - 1:
                            pump(bi * TPB + j)
                        want = len(chunks) * (j + 1) // TPB
                        while done < want:
                            chunks[done]()
                            done += 1
                # the LAST tile is gathered in two halves so the second
                # half's transfer overlaps the first half's compute,
                # shortening the drain tail. Each half lands in the low
                # half of its own full-size ring tile.
                K2 = K // 2
                IDXGl, W8dl = blk[(NT - 1) // TPB]
                tl = (NT - 1) % TPB
                halves = []
                for h in range(2):
                    Gh = gpool.tile([P, K, ES], F16, tag="G",
                                    name=f"G15{h}")
                    c0 = tl * K * 8 + h * K2 * 8
                    nc.gpsimd.dma_gather(
                        Gh[:, 0:K2], img[BPC - 1],
                        IDXGl[:, c0:c0 + K2 * 8],
                        P * K2, P * K2, ES, single_packet=False,
                    )
                    halves.append(Gh)
                for ta, Ga in pending:
                    process_eng(ta, Ga, 0, 51, nc.vector)
                    process_eng(ta, Ga, 51, K, nc.gpsimd)
                for h, Gh in enumerate(halves):
                    sl = slice((NT - 1) * K + h * K2,
                               (NT - 1) * K + (h + 1) * K2)
                    G4 = Gh[:, 0:K2].rearrange("p k (x c) -> p k x c", c=C)
                    O = opool.tile([P, K, C], F16, tag="O",
                                   name=f"O15{h}")
                    for lo, hi, eng in ((0, 26, nc.vector),
                                        (26, K2, nc.gpsimd)):
                        kr = slice(lo, hi)
                        wsl = slice(tl * K + h * K2 + lo,
                                    tl * K + h * K2 + hi)
                        g4 = G4[:, kr]
                        gm = g4.rearrange("p k x (h2 d) -> p (k x) h2 d",
                                          d=2)
                        wm = W8dl[:, wsl].rearrange("p k x d -> p (k x) d")
                        wm = wm.unsqueeze(2).broadcast_to(
                            [P, (hi - lo) * 8, C // 2, 2])
                        eng.tensor_tensor(out=gm, in0=gm, in1=wm,
                                          op=Alu.mult)
                        eng.tensor_tensor(out=g4[:, :, 0:3, :],
                                          in0=g4[:, :, 0:3, :],
                                          in1=g4[:, :, 4:7, :], op=Alu.add)
                        eng.tensor_tensor(out=g4[:, :, 0, :],
                                          in0=g4[:, :, 0, :],
                                          in1=g4[:, :, 1, :], op=Alu.add)
                        eng.tensor_tensor(out=O[:, kr],
                                          in0=g4[:, :, 0, :],
                                          in1=g4[:, :, 2, :], op=Alu.add)
                        nc.sync.dma_start(out=out_v[:, sl, :][:, kr],
                                          in_=O[:, kr])

    nc.compile()
    return nc


_CACHED = {}


def _get_program():
    if "nc" not in _CACHED:
        _CACHED["nc"] = _build_program()
    return _CACHED["nc"]


def _linspace_f32(n):
    # match jnp.linspace(-1, 1, n, dtype=float32): iota*step + start in f32
    step = np.float32(2.0) / np.float32(n - 1)
    return (np.arange(n, dtype=np.float32) * step + np.float32(-1.0)).astype(
        np.float32)


def _host_tables():
    # pixel (p, s): per-core pixel id g = s*128 + p; within-image id
    # pid = g % IMG_PIX; j = pid % W = p + 128*(s % 2);
    # i = pid // W = (s % HS) // 2
    xs = _linspace_f32(W)
    ys = _linspace_f32(H)
    xbt = np.stack([xs[:P], xs[P:2 * P]], axis=1)          # [P, 2]
    ybt = np.broadcast_to(ys[:HS // 2], (P, HS // 2))      # [P, 256]
    return np.ascontiguousarray(xbt), np.ascontiguousarray(ybt)


def _repack_images(image16):
    # image16: [nb, H, W, C] fp16 -> per image entries
    # [par, hp, u, r, c4, ch] = Ipad[2*hp + par + r, 2*u + c, ch]
    nb = image16.shape[0]
    pad = np.zeros((nb, H + 2, W + 2, C), np.float16)
    pad[:, :H, :W, :] = image16
    sb, sh, sw, sc = pad.strides
    ev = np.lib.stride_tricks.as_strided(
        pad,
        shape=(nb, 2, H // 2, W // 2, 2, 4, C),
        strides=(sb, sh, 2 * sh, 2 * sw, sh, sw, sc),
    )
    return np.ascontiguousarray(ev).reshape(nb, NE, ES)


def kernel(image: np.ndarray, flow: np.ndarray) -> np.ndarray:
    image = np.asarray(image)
    flow = np.asarray(flow)
    assert image.shape == (B, H, W, C) and flow.shape == (B, H, W, 2)

    nc = _get_program()
    xbt, ybt = _host_tables()

    entries = _repack_images(image.astype(np.float16))  # [B, NE, ES]
    flow32 = np.ascontiguousarray(flow, dtype=np.float32).reshape(
        NCORES, SLOTS, P, 2).transpose(0, 2, 1, 3).reshape(NCORES, NPIX, 2)

    in_maps = []
    for c in range(NCORES):
        in_maps.append({
            "img": np.ascontiguousarray(entries[BPC * c:BPC * (c + 1)]),
            "flow": np.ascontiguousarray(flow32[c]),
            "xbt": xbt,
            "ybt": ybt,
        })

    res = run_bass_kernel_spmd(nc, in_maps, list(range(NCORES)))
    _CACHED["last_result"] = res
    outs = [res.results[c]["out"].reshape(P, SLOTS, C).transpose(1, 0, 2)
            .reshape(NPIX, C) for c in range(NCORES)]
    return np.concatenate(outs, axis=0).reshape(B, H, W, C).astype(np.float32)
